# revision 8
# baseline (speedup 1.0000x reference)
"""AttentionHead kernel for 8 TRN2 NeuronCores.

Problem: q = x@Wq+bq; k = y@Wk+bk; v = y@Wv+bv
         att = softmax(q k^T / sqrt(128));  att = triu(att, k=1)  (AFTER softmax)
         out = att @ v
Shapes: x [4, 2048, 1024], y [4, 2048, 1024], W* [1024, 128], out [4, 2048, 128].

Sharding: 8 cores = (batch b in 0..3) x (query-half h in 0..1). Core (b, h)
computes queries [h*1024, (h+1)*1024) of batch b against all 2048 keys.
No cross-core communication.

SPMD uniformity trick: the post-softmax causal mask (keep key j > query i)
depends on the query offset h*1024, which differs per core, but all cores
must run the SAME graph. We rotate the key axis per core on host
(j_local = (j_global - h*1024) mod 2048). Then for every core:
  - keys j_local in [0, 1024): keep iff j_local > i_local  (same triangular
    band for every core -> one compile-time mask input shared by all cores)
  - keys j_local in [1024, 2048): keep-all for h=0, drop-all for h=1 ->
    handled by scaling those V tiles by a per-core scalar g in {1.0, 0.0}.
The softmax normalizer sums exp over ALL keys (mask comes after softmax),
and is invariant to the key rotation.

On-chip layout: host pre-transposes x/y to [feature, seq] bf16 so the
projections produce qT [d, i] / kT [d, j] / v [j, d] directly in the layouts
the PE array needs. Scores are computed transposed, ST [j, i]; Z[i] =
sum_j exp(ST) via a ones-stationary matmul (broadcast over partitions);
O^T [d, i] = sum_j v[j, d]^T . maskedexp[j, i]; final scale by 1/Z on DVE.
Host transposes O^T back.
"""

import numpy as np
import ml_dtypes

B = 4
LQ = 2048
LK = 2048
XS = 1024
PD = 128
LQS = LQ // 2  # queries per core: 1024

NE = XS // 128  # 8 contraction tiles for projections
NT = LK // 128  # 16 key tiles
CH = 512  # query chunk (PSUM bank = 512 f32)
NCH = LQS // CH  # 2 chunks
SM_SCALE = 1.0 / float(np.sqrt(PD))

_BF16 = ml_dtypes.bfloat16

_graph_cache = {}


def _build_graph(apply_mask: bool):
    import concourse.mybir as mybir
    from concourse import bacc
    from concourse.tile import TileContext

    BF = mybir.dt.bfloat16
    F32 = mybir.dt.float32
    Exp = mybir.ActivationFunctionType.Exp
    Identity = mybir.ActivationFunctionType.Identity

    nc = bacc.Bacc()

    xT = nc.declare_dram_parameter("xT", [XS, LQS], BF, isOutput=False)
    yT = nc.declare_dram_parameter("yT", [XS, LK], BF, isOutput=False)
    Wq = nc.declare_dram_parameter("Wq", [XS, PD], BF, isOutput=False)
    Wk = nc.declare_dram_parameter("Wk", [XS, PD], BF, isOutput=False)
    Wv = nc.declare_dram_parameter("Wv", [XS, PD], BF, isOutput=False)
    bq = nc.declare_dram_parameter("bq", [PD, 1], F32, isOutput=False)
    bk = nc.declare_dram_parameter("bk", [PD, 1], F32, isOutput=False)
    # v tiles are [j, d]: bv lives on the free axis, so it is folded into the
    # projection matmul as a K=1 accumulation (ones-column x bv row).
    bv = nc.declare_dram_parameter("bv", [1, PD], BF, isOutput=False)
    # Per-v-tile scale g (cols 0..7 = 1; cols 8..15 = 1 or 0 per core),
    # broadcast over the 128 partitions.
    gv = nc.declare_dram_parameter("gv", [128, NT], F32, isOutput=False)
    if apply_mask:
        # tri[jj, c] = 1.0 if jj > c - 384 else 0.0, c in [0, 896).
        # Band mask for key-tile t vs query chunk c0: delta = 128*t - 512*c0,
        # slice cols [384-delta, 384-delta+512).
        tri = nc.declare_dram_parameter("tri", [128, 896], BF, isOutput=False)
    out_ext = nc.declare_dram_parameter("out", [PD, LQS], F32, isOutput=True)

    with TileContext(nc) as tc:
        with (
            tc.tile_pool(name="const", bufs=1) as const_pool,
            tc.tile_pool(name="sb", bufs=1) as sb_pool,
            tc.tile_pool(name="exp", bufs=3) as exp_pool,
            tc.tile_pool(name="ps", bufs=2, space="PSUM") as ps_pool,
            tc.tile_pool(name="psacc", bufs=1, space="PSUM") as psacc_pool,
        ):
            # ---- load inputs ----
            xT_sb = sb_pool.tile([128, NE, LQS], BF)
            yT_sb = sb_pool.tile([128, NE, LK], BF)
            Wq_sb = sb_pool.tile([128, NE, PD], BF)
            Wk_sb = sb_pool.tile([128, NE, PD], BF)
            Wv_sb = sb_pool.tile([128, NE, PD], BF)
            for e in range(NE):
                nc.sync.dma_start(out=xT_sb[:, e, :], in_=xT[e * 128:(e + 1) * 128, :])
                nc.sync.dma_start(out=yT_sb[:, e, :], in_=yT[e * 128:(e + 1) * 128, :])
                nc.sync.dma_start(out=Wq_sb[:, e, :], in_=Wq[e * 128:(e + 1) * 128, :])
                nc.sync.dma_start(out=Wk_sb[:, e, :], in_=Wk[e * 128:(e + 1) * 128, :])
                nc.sync.dma_start(out=Wv_sb[:, e, :], in_=Wv[e * 128:(e + 1) * 128, :])
            bq_sb = const_pool.tile([128, 1], F32)
            bk_sb = const_pool.tile([128, 1], F32)
            bv_sb = const_pool.tile([1, PD], BF)
            gv_sb = const_pool.tile([128, NT], F32)
            nc.sync.dma_start(out=bq_sb, in_=bq[:, :])
            nc.sync.dma_start(out=bk_sb, in_=bk[:, :])
            nc.sync.dma_start(out=bv_sb, in_=bv[:, :])
            nc.sync.dma_start(out=gv_sb, in_=gv[:, :])
            if apply_mask:
                tri_sb = const_pool.tile([128, 896], BF)
                nc.sync.dma_start(out=tri_sb, in_=tri[:, :])
            ones_sb = const_pool.tile([128, 128], BF)
            nc.vector.memset(ones_sb, 1.0)

            # ---- projections ----
            # qT [d, i] = sum_e Wq[e, d]^T xT[e, i]  (+ bq per-partition)
            qT_sb = sb_pool.tile([128, LQS], BF)
            for c in range(NCH):
                ps = ps_pool.tile([128, CH], mybir.dt.float32, tag="proj")
                for e in range(NE):
                    nc.tensor.matmul(
                        ps,
                        lhsT=Wq_sb[:, e, :],
                        rhs=xT_sb[:, e, c * CH:(c + 1) * CH],
                        start=(e == 0),
                        stop=(e == NE - 1),
                    )
                nc.scalar.activation(
                    qT_sb[:, c * CH:(c + 1) * CH], ps, Identity, bias=bq_sb[:, 0:1]
                )
            # kT [d, j]
            kT_sb = sb_pool.tile([128, LK], BF)
            for c in range(LK // CH):
                ps = ps_pool.tile([128, CH], mybir.dt.float32, tag="proj")
                for e in range(NE):
                    nc.tensor.matmul(
                        ps,
                        lhsT=Wk_sb[:, e, :],
                        rhs=yT_sb[:, e, c * CH:(c + 1) * CH],
                        start=(e == 0),
                        stop=(e == NE - 1),
                    )
                nc.scalar.activation(
                    kT_sb[:, c * CH:(c + 1) * CH], ps, Identity, bias=bk_sb[:, 0:1]
                )
            # v tiles [j, d], scaled by g and biased bv*g on the way out of PSUM
            v_sb = sb_pool.tile([128, NT, PD], BF)
            for t in range(NT):
                ps = ps_pool.tile([128, PD], mybir.dt.float32, tag="proj")
                for e in range(NE):
                    nc.tensor.matmul(
                        ps,
                        lhsT=yT_sb[:, e, t * 128:(t + 1) * 128],
                        rhs=Wv_sb[:, e, :],
                        start=(e == 0),
                        stop=False,
                    )
                # += 1 * bv[d]  (K=1 bias fold)
                nc.tensor.matmul(
                    ps,
                    lhsT=ones_sb[0:1, :],
                    rhs=bv_sb[:, :],
                    start=False,
                    stop=True,
                )
                # scale whole tile (v + bv) by g_t in {0, 1}
                nc.scalar.activation(
                    v_sb[:, t, :], ps, Identity, scale=gv_sb[:, t:t + 1]
                )

            # ---- attention ----
            z_ps = [
                psacc_pool.tile(
                    [128, CH], mybir.dt.float32, tag=f"z{c}", name=f"z_ps{c}"
                )
                for c in range(NCH)
            ]
            o_ps = [
                psacc_pool.tile(
                    [128, CH], mybir.dt.float32, tag=f"o{c}", name=f"o_ps{c}"
                )
                for c in range(NCH)
            ]
            for t in range(NT):
                e_sb = exp_pool.tile([128, LQS], BF)
                for c in range(NCH):
                    cs = slice(c * CH, (c + 1) * CH)
                    st = ps_pool.tile([128, CH], mybir.dt.float32, tag="st")
                    # ST [j, i] = kT_t^T qT  (full d contraction in one shot)
                    nc.tensor.matmul(
                        st,
                        lhsT=kT_sb[:, t * 128:(t + 1) * 128],
                        rhs=qT_sb[:, cs],
                        start=True,
                        stop=True,
                    )
                    nc.scalar.activation(e_sb[:, cs], st, Exp, scale=SM_SCALE)
                    # Z += ones^T @ exp  (broadcast row-sum over partitions)
                    nc.tensor.matmul(
                        z_ps[c],
                        lhsT=ones_sb,
                        rhs=e_sb[:, cs],
                        start=(t == 0),
                        stop=(t == NT - 1),
                    )
                    if apply_mask and t // 4 == c:
                        off = 384 - (128 * t - CH * c)
                        nc.vector.tensor_mul(
                            e_sb[:, cs], e_sb[:, cs], tri_sb[:, off:off + CH]
                        )
                    # O^T [d, i] += v_t^T @ maskedexp ; skip all-zero tiles
                    if (not apply_mask) or t >= 4 * c:
                        first_t = 4 * c if apply_mask else 0
                        nc.tensor.matmul(
                            o_ps[c],
                            lhsT=v_sb[:, t, :],
                            rhs=e_sb[:, cs],
                            start=(t == first_t),
                            stop=(t == NT - 1),
                        )

            # ---- normalize + store ----
            recip_sb = sb_pool.tile([128, LQS], mybir.dt.float32)
            out_sb = sb_pool.tile([128, LQS], mybir.dt.float32)
            for c in range(NCH):
                cs = slice(c * CH, (c + 1) * CH)
                nc.vector.reciprocal(recip_sb[:, cs], z_ps[c])
                nc.vector.tensor_mul(out_sb[:, cs], o_ps[c], recip_sb[:, cs])
                nc.sync.dma_start(out=out_ext[:, cs], in_=out_sb[:, cs])

    nc.finalize()
    return nc


def _get_graph(apply_mask: bool):
    key = bool(apply_mask)
    if key not in _graph_cache:
        _graph_cache[key] = _build_graph(key)
    return _graph_cache[key]


def kernel(**inputs) -> np.ndarray:
    from concourse.bass_utils import run_bass_kernel_spmd

    x = np.asarray(inputs["x"], dtype=np.float32)
    y = np.asarray(inputs["y"], dtype=np.float32)
    Wq = np.asarray(inputs["Wq"], dtype=np.float32)
    Wk = np.asarray(inputs["Wk"], dtype=np.float32)
    Wv = np.asarray(inputs["Wv"], dtype=np.float32)
    bq = np.asarray(inputs["bq"], dtype=np.float32)
    bk = np.asarray(inputs["bk"], dtype=np.float32)
    bv = np.asarray(inputs["bv"], dtype=np.float32)
    mask = bool(np.asarray(inputs["mask"]).item())

    nc = _get_graph(mask)

    Wq_b = Wq.astype(_BF16)
    Wk_b = Wk.astype(_BF16)
    Wv_b = Wv.astype(_BF16)
    bq_c = np.ascontiguousarray(bq.reshape(PD, 1))
    bk_c = np.ascontiguousarray(bk.reshape(PD, 1))

    if mask:
        cc = np.arange(896, dtype=np.int64)[None, :] - 384
        jj = np.arange(128, dtype=np.int64)[:, None]
        tri = (jj > cc).astype(_BF16)

    in_maps = []
    for core in range(8):
        b, h = core // 2, core % 2
        qoff = h * LQS
        xs = x[b, qoff:qoff + LQS, :]
        ys = np.roll(y[b], -qoff, axis=0) if qoff else y[b]
        g = 1.0 if (h == 0 or not mask) else 0.0
        gcols = np.ones(NT, dtype=np.float32)
        gcols[NT // 2:] = g
        gv_arr = np.broadcast_to(gcols, (128, NT)).copy()
        m = {
            "xT": np.ascontiguousarray(xs.T).astype(_BF16),
            "yT": np.ascontiguousarray(ys.T).astype(_BF16),
            "Wq": Wq_b,
            "Wk": Wk_b,
            "Wv": Wv_b,
            "bq": bq_c,
            "bk": bk_c,
            "bv": np.ascontiguousarray(bv.reshape(1, PD)).astype(_BF16),
            "gv": gv_arr,
        }
        if mask:
            m["tri"] = tri
        in_maps.append(m)

    res = run_bass_kernel_spmd(nc, in_maps, core_ids=list(range(8)))

    out = np.empty((B, LQ, PD), dtype=np.float32)
    for core in range(8):
        b, h = core // 2, core % 2
        qoff = h * LQS
        out[b, qoff:qoff + LQS, :] = res.results[core]["out"].T
    return out
